# revision 3
# baseline (speedup 1.0000x reference)
"""Trainium2 Bass kernel for CompositionalCodebookLayer (vector-quantization).

Reference computation (per token t of B*S=8192, per codebook c of 16):
    idx[t,c]  = argmin_n || x[t, c*64:(c+1)*64] - codebook[c, n, :] ||^2 ,  n in [0, 2048)
    out[t, c*64:(c+1)*64] = codebook[c, idx[t,c], :]

Device strategy (data-parallel over tokens, 8 cores x 1024 tokens):
  - argmin of distance == argmax of (dot(x_c, cb_c[n]) - 0.5*||cb_c[n]||^2).
    The -0.5*||cb||^2 term is folded into the matmul as a 65th contraction
    row (x side gets a constant 1.0 row), so PE emits final scores directly.
  - Host pre-transposes x and the codebook so contraction (sub-dim) lies on
    SBUF partitions; no on-device transposes at all.
  - Per (128-token tile, codebook): PE computes scores [128, 2048] into PSUM
    (4 matmuls of N=512), DVE max + max_index give the exact fp32 argmax with
    first-occurrence tie semantics (matches jnp.argmin ties), and an indirect
    DMA gathers the winning code vectors from HBM into the output tile.
"""

import numpy as np

B, S, D = 4, 2048, 1024
C, N, SUB = 16, 2048, 64
NCORES = 8
TOK = B * S              # 8192 tokens
TPC = TOK // NCORES      # 1024 tokens per core
P = 128                  # tokens per tile (partition dim)
NTILES = TPC // P        # 8 tiles per core
KDIM = SUB + 1           # 64 sub-dims + 1 bias row
NCHUNK = 512             # fp32 matmul moving-operand max (one PSUM bank)

_CACHE = {}


def _build_program():
    import concourse.bacc as bacc
    import concourse.mybir as mybir
    import concourse.tile as tile
    from concourse.bass import IndirectOffsetOnAxis

    f32 = mybir.dt.float32

    nc = bacc.Bacc(
        "TRN2",
        target_bir_lowering=False,
        debug=False,
        enable_asserts=False,
        num_devices=NCORES,
    )

    xt_d = nc.dram_tensor("xt", [C, KDIM, TPC], f32, kind="ExternalInput").ap()
    cbt_d = nc.dram_tensor("cbt", [C, KDIM, N], f32, kind="ExternalInput").ap()
    cb_d = [
        nc.dram_tensor(f"cb{c}", [N, SUB], f32, kind="ExternalInput").ap()
        for c in range(C)
    ]
    y_d = nc.dram_tensor("y", [TPC, D], f32, kind="ExternalOutput").ap()

    with tile.TileContext(nc) as tc:
        with (
            tc.tile_pool(name="cbt", bufs=1) as cbt_pool,
            tc.tile_pool(name="xt", bufs=2) as xt_pool,
            tc.tile_pool(name="out", bufs=2) as out_pool,
            tc.tile_pool(name="small", bufs=8) as small_pool,
            tc.tile_pool(name="psum", bufs=2, space="PSUM") as psum_pool,
        ):
            # Codebook (transposed + bias row), resident in SBUF for the
            # whole kernel: [65, 16*2048] f32.
            cbt_sb = cbt_pool.tile([KDIM, C * N], f32)
            nc.sync.dma_start(
                cbt_sb[:].rearrange("p (c n) -> p c n", c=C),
                cbt_d.rearrange("c p n -> p c n"),
            )

            for t in range(NTILES):
                # x^T slice for these 128 tokens, all codebooks: [65, 16*128]
                xt_sb = xt_pool.tile([KDIM, C * P], f32)
                nc.sync.dma_start(
                    xt_sb[:].rearrange("p (c t) -> p c t", c=C),
                    xt_d[:, :, t * P : (t + 1) * P].rearrange("c p t -> p c t"),
                )
                out_sb = out_pool.tile([P, D], f32)
                for c in range(C):
                    ps = psum_pool.tile([P, N], f32)
                    for k in range(N // NCHUNK):
                        nc.tensor.matmul(
                            ps[:, k * NCHUNK : (k + 1) * NCHUNK],
                            lhsT=xt_sb[:, c * P : (c + 1) * P],
                            rhs=cbt_sb[:, c * N + k * NCHUNK : c * N + (k + 1) * NCHUNK],
                            start=True,
                            stop=True,
                        )
                    mx8 = small_pool.tile([P, 8], f32, tag="mx8")
                    ix8 = small_pool.tile([P, 8], mybir.dt.uint32, tag="ix8")
                    nc.vector.max(out=mx8[:], in_=ps[:])
                    nc.vector.max_index(out=ix8[:], in_max=mx8[:], in_values=ps[:])
                    nc.gpsimd.indirect_dma_start(
                        out=out_sb[:, c * SUB : (c + 1) * SUB],
                        out_offset=None,
                        in_=cb_d[c][:],
                        in_offset=IndirectOffsetOnAxis(ap=ix8[:, :1], axis=0),
                    )
                nc.sync.dma_start(y_d[t * P : (t + 1) * P, :], out_sb[:])

    nc.compile()
    return nc


def _host_prep(x, codebook):
    x = np.ascontiguousarray(x, dtype=np.float32)
    cb = np.ascontiguousarray(codebook, dtype=np.float32)
    xr = x.reshape(TOK, C, SUB)

    cbt = np.empty((C, KDIM, N), dtype=np.float32)
    cbt[:, :SUB, :] = cb.transpose(0, 2, 1)
    cbt[:, SUB, :] = -0.5 * np.square(cb).sum(-1)

    cb_ins = {f"cb{c}": np.ascontiguousarray(cb[c]) for c in range(C)}

    in_maps = []
    for i in range(NCORES):
        shard = xr[i * TPC : (i + 1) * TPC]          # [TPC, C, SUB]
        xt = np.empty((C, KDIM, TPC), dtype=np.float32)
        xt[:, :SUB, :] = shard.transpose(1, 2, 0)
        xt[:, SUB, :] = 1.0
        in_maps.append({"xt": xt, "cbt": cbt, **cb_ins})
    return in_maps


def kernel(x, codebook, _trace=False):
    from concourse.bass_utils import run_bass_kernel_spmd

    if "nc" not in _CACHE:
        _CACHE["nc"] = _build_program()
    nc = _CACHE["nc"]

    in_maps = _host_prep(np.asarray(x), np.asarray(codebook))
    res = run_bass_kernel_spmd(
        nc, in_maps, core_ids=list(range(NCORES)), trace=_trace
    )
    _CACHE["last_result"] = res
    y = np.concatenate([r["y"] for r in res.results], axis=0)  # [TOK, D]
    return y.reshape(B, S, D)
